# revision 5
# baseline (speedup 1.0000x reference)
"""MoE (top-2 of 8 experts) Trainium2 kernel.

Strategy: expert-parallel across the 8 NeuronCores (1 expert per core).
The router (0.1% of the FLOPs) runs on the host with the exact op
sequence of the reference so top-k decisions match bitwise; the host
gathers each expert's tokens, each core runs that expert's FFN over its
tokens (feature-major layout, float32r matmuls), and the host
scatter-adds the per-expert outputs back into the full output.

Weights are host-repacked so every weight DMA is per-partition
contiguous (few large DMAs instead of thousands of 64KB ones), split
across both HWDGE queues (SP and Activation).
"""

import numpy as np

D_MODEL = 1024
D_FF = 4096
NUM_EXPERTS = 8
TOP_K = 2
N_CORES = 8

KD = D_MODEL // 128  # 8 partition-tiles along d_model
KF = D_FF // 128  # 32 partition-tiles along d_ff

_PROGRAM_CACHE = {}


def _build_program(C, T_super):
    from contextlib import ExitStack

    import concourse.bacc as bacc
    import concourse.mybir as mybir
    import concourse.tile as tile

    f32 = mybir.dt.float32
    f32r = mybir.dt.float32r
    Relu = mybir.ActivationFunctionType.Relu
    add = mybir.AluOpType.add
    mult = mybir.AluOpType.mult

    assert C % T_super == 0
    n_super = C // T_super
    T_half = T_super // 2
    assert T_half >= 256, "float32r needs moving free dim >= 256 for full rate"

    nc = bacc.Bacc("TRN2", target_bir_lowering=False, debug=False, num_devices=N_CORES)
    xT = nc.dram_tensor("xT", [D_MODEL, C], f32r, kind="ExternalInput").ap()
    # w1 packed as [p, f, k]: w1p[p, f, k] = w1[k*128+p, f]
    w1p = nc.dram_tensor("w1p", [128, D_FF, KD], f32r, kind="ExternalInput").ap()
    b1 = nc.dram_tensor("b1", [D_FF], f32, kind="ExternalInput").ap()
    # w2 packed as [p, kh, d, kl]: w2p[p, kh, d, kl] = w2[(kh*16+kl)*128+p, d]
    w2p = nc.dram_tensor("w2p", [128, 2, D_MODEL, KF // 2], f32r, kind="ExternalInput").ap()
    b2 = nc.dram_tensor("b2", [D_MODEL], f32, kind="ExternalInput").ap()
    g = nc.dram_tensor("g", [C], f32, kind="ExternalInput").ap()
    yT = nc.dram_tensor("yT", [D_MODEL, C], f32, kind="ExternalOutput").ap()

    with tile.TileContext(nc) as tc, ExitStack() as ctx:
        const = ctx.enter_context(tc.tile_pool(name="const", bufs=1))
        xpool = ctx.enter_context(tc.tile_pool(name="x", bufs=KD + 1))
        hpool = ctx.enter_context(tc.tile_pool(name="h", bufs=KF))
        w1pool = ctx.enter_context(tc.tile_pool(name="w1pl", bufs=4))
        w2pool = ctx.enter_context(tc.tile_pool(name="w2pl", bufs=4))
        opool = ctx.enter_context(tc.tile_pool(name="o", bufs=4))
        ps1 = ctx.enter_context(tc.tile_pool(name="ps1", bufs=4, space="PSUM"))
        ps2 = ctx.enter_context(tc.tile_pool(name="ps2", bufs=4, space="PSUM"))

        # Constants: b1 as [128, 32] (col f = b1[f*128:(f+1)*128]), b2 as
        # [128, 8], g broadcast to all 128 partitions.
        b1_sb = const.tile([128, KF], f32)
        nc.sync.dma_start(b1_sb[:], b1.rearrange("(f p) -> p f", p=128))
        b2_sb = const.tile([128, KD], f32)
        nc.sync.dma_start(b2_sb[:], b2.rearrange("(d p) -> p d", p=128))
        g_sb = const.tile([128, C], f32)
        nc.sync.dma_start(g_sb[:], g.unsqueeze(0).partition_broadcast(128))

        for S in range(n_super):
            tsl = slice(S * T_super, (S + 1) * T_super)
            xs = []
            for k in range(KD):
                xt = xpool.tile([128, T_super], f32r, tag="x", name=f"x_{S}_{k}")
                nc.sync.dma_start(xt[:], xT[k * 128 : (k + 1) * 128, tsl])
                xs.append(xt)

            # ---- matmul 1: h[f, t] = relu(sum_k w1[k,f]^T x[k,t] + b1[f])
            hs = []
            for f in range(KF):
                w1c = w1pool.tile([128, 128, KD], f32r, tag="w1c", name=f"w1c_{S}_{f}")
                nc.sync.dma_start(w1c[:], w1p[:, f * 128 : (f + 1) * 128, :])
                pst = [
                    ps1.tile([128, T_half], f32, tag="ps1", name=f"ps1_{S}_{f}_{m}")
                    for m in range(2)
                ]
                for k in range(KD):
                    for m in range(2):
                        nc.tensor.matmul(
                            pst[m][:],
                            w1c[:, :, k],
                            xs[k][:, m * T_half : (m + 1) * T_half],
                            start=(k == 0),
                            stop=(k == KD - 1),
                        )
                ht = hpool.tile([128, T_super], f32r, tag="h", name=f"h_{S}_{f}")
                for m in range(2):
                    nc.scalar.activation(
                        ht[:, m * T_half : (m + 1) * T_half],
                        pst[m][:],
                        Relu,
                        bias=b1_sb[:, f : f + 1],
                        scale=1.0,
                    )
                hs.append(ht)

            # ---- matmul 2: y[d, t] = (sum_f w2[f,d]^T h[f,t] + b2[d]) * g[t]
            for d in range(KD):
                w2cs = []
                for kh in range(2):
                    w2c = w2pool.tile(
                        [128, 128, KF // 2], f32r, tag="w2c", name=f"w2c_{S}_{d}_{kh}"
                    )
                    nc.scalar.dma_start(w2c[:], w2p[:, kh, d * 128 : (d + 1) * 128, :])
                    w2cs.append(w2c)
                pst = [
                    ps2.tile([128, T_half], f32, tag="ps2", name=f"ps2_{S}_{d}_{m}")
                    for m in range(2)
                ]
                for kf in range(KF):
                    kh, kl = divmod(kf, KF // 2)
                    for m in range(2):
                        nc.tensor.matmul(
                            pst[m][:],
                            w2cs[kh][:, :, kl],
                            hs[kf][:, m * T_half : (m + 1) * T_half],
                            start=(kf == 0),
                            stop=(kf == KF - 1),
                        )
                ot = opool.tile([128, T_super], f32, tag="o", name=f"o_{S}_{d}")
                for m in range(2):
                    nc.vector.scalar_tensor_tensor(
                        ot[:, m * T_half : (m + 1) * T_half],
                        pst[m][:],
                        b2_sb[:, d : d + 1],
                        g_sb[:, S * T_super + m * T_half : S * T_super + (m + 1) * T_half],
                        op0=add,
                        op1=mult,
                    )
                nc.scalar.dma_start(yT[d * 128 : (d + 1) * 128, tsl], ot[:])

    nc.compile()
    return nc


def _get_program(C, T_super):
    key = (C, T_super)
    if key not in _PROGRAM_CACHE:
        _PROGRAM_CACHE[key] = _build_program(C, T_super)
    return _PROGRAM_CACHE[key]


def _route_host(x, gate_w, gate_b):
    """Router math mirroring the reference op-for-op (same jax eager ops in
    the same order -> matching top-k decisions)."""
    import jax
    import jax.numpy as jnp

    router_logits = jnp.einsum("bsd,de->bse", x, gate_w) + gate_b
    router_probs = jax.nn.softmax(router_logits, axis=-1)
    topk_probs, topk_idx = jax.lax.top_k(router_probs, TOP_K)
    topk_probs = topk_probs / jnp.sum(topk_probs, axis=-1, keepdims=True)

    one_hot = jax.nn.one_hot(topk_idx, NUM_EXPERTS, dtype=x.dtype)
    expert_usage = jnp.mean(one_hot, axis=(0, 1, 2))
    avg_probs = jnp.mean(router_probs, axis=(0, 1))
    load_balance_loss = jnp.var(avg_probs, ddof=1)
    return (
        np.asarray(topk_probs),
        np.asarray(topk_idx),
        np.asarray(expert_usage),
        np.asarray(load_balance_loss),
    )


def _pick_tiling(C_raw):
    T_super = 768
    n_super = max(1, -(-C_raw // T_super))
    return n_super * T_super, T_super


def _prepare_dispatch(x, topk_probs, topk_idx):
    """Per-expert gathered rows and gains."""
    x_flat = np.ascontiguousarray(np.asarray(x, dtype=np.float32).reshape(-1, D_MODEL))
    tk_idx = topk_idx.reshape(-1, TOP_K)
    tk_p = topk_probs.reshape(-1, TOP_K).astype(np.float32)

    rows_per_e = []
    gains_per_e = []
    for e in range(NUM_EXPERTS):
        sel = tk_idx == e
        rows = np.nonzero(sel.any(axis=1))[0]
        gains = (tk_p * sel)[rows].sum(axis=1, dtype=np.float32)
        rows_per_e.append(rows)
        gains_per_e.append(gains)
    return x_flat, rows_per_e, gains_per_e


def _build_in_maps(x_flat, rows_per_e, gains_per_e, w1, b1, w2, b2, C):
    w1_np = np.asarray(w1, dtype=np.float32)
    b1_np = np.asarray(b1, dtype=np.float32)
    w2_np = np.asarray(w2, dtype=np.float32)
    b2_np = np.asarray(b2, dtype=np.float32)

    in_maps = []
    for e in range(NUM_EXPERTS):
        rows = rows_per_e[e]
        xg = np.zeros((C, D_MODEL), dtype=np.float32)
        xg[: len(rows)] = x_flat[rows]
        gg = np.zeros((C,), dtype=np.float32)
        gg[: len(rows)] = gains_per_e[e]
        # w1p[p, f, k] = w1[k*128+p, f]
        w1p = np.ascontiguousarray(
            w1_np[e].reshape(KD, 128, D_FF).transpose(1, 2, 0)
        )
        # w2p[p, kh, d, kl] = w2[(kh*16+kl)*128+p, d]
        w2p = np.ascontiguousarray(
            w2_np[e].reshape(2, KF // 2, 128, D_MODEL).transpose(2, 0, 3, 1)
        )
        in_maps.append(
            {
                "xT": np.ascontiguousarray(xg.T),
                "w1p": w1p,
                "b1": b1_np[e],
                "w2p": w2p,
                "b2": b2_np[e],
                "g": gg,
            }
        )
    return in_maps


def kernel(x, gate_w, gate_b, w1, b1, w2, b2):
    from concourse.bass_utils import run_bass_kernel_spmd

    topk_probs, topk_idx, expert_usage, load_balance_loss = _route_host(
        x, gate_w, gate_b
    )
    x_flat, rows_per_e, gains_per_e = _prepare_dispatch(x, topk_probs, topk_idx)

    C_raw = max(len(r) for r in rows_per_e)
    C, T_super = _pick_tiling(C_raw)
    nc = _get_program(C, T_super)

    in_maps = _build_in_maps(x_flat, rows_per_e, gains_per_e, w1, b1, w2, b2, C)
    res = run_bass_kernel_spmd(nc, in_maps, list(range(N_CORES)))

    B, S, _ = np.asarray(x).shape
    out_flat = np.zeros((B * S, D_MODEL), dtype=np.float32)
    for e in range(NUM_EXPERTS):
        rows = rows_per_e[e]
        yT = res.results[e]["yT"]
        out_flat[rows] += yT[:, : len(rows)].T

    return (
        out_flat.reshape(B, S, D_MODEL),
        expert_usage.astype(np.float32),
        np.float32(load_balance_loss),
    )


# revision 7
# speedup vs baseline: 9.6794x; 9.6794x over previous
"""MoE (top-2 of 8 experts) Trainium2 kernel.

Strategy: expert-parallel across the 8 NeuronCores (1 expert per core).
The router (0.1% of the FLOPs) runs on the host with the exact op
sequence of the reference so top-k decisions match bitwise; the host
gathers each expert's tokens, each core runs that expert's FFN over its
tokens (feature-major layout, float32r matmuls), and the host
scatter-adds the per-expert outputs back into the full output.

Weights are host-repacked so every weight DMA is per-partition
contiguous (few large DMAs instead of thousands of 64KB ones), split
across both HWDGE queues (SP and Activation).
"""

import numpy as np

D_MODEL = 1024
D_FF = 4096
NUM_EXPERTS = 8
TOP_K = 2
N_CORES = 8

KD = D_MODEL // 128  # 8 partition-tiles along d_model
KF = D_FF // 128  # 32 partition-tiles along d_ff

_PROGRAM_CACHE = {}


def _build_program(C, T_super, reps=1):
    from contextlib import ExitStack

    import concourse.bacc as bacc
    import concourse.mybir as mybir
    import concourse.tile as tile

    f32 = mybir.dt.float32
    f32r = mybir.dt.float32r
    Relu = mybir.ActivationFunctionType.Relu
    add = mybir.AluOpType.add
    mult = mybir.AluOpType.mult

    assert C % T_super == 0
    n_super = C // T_super
    T_half = T_super // 2
    assert T_half >= 256, "float32r needs moving free dim >= 256 for full rate"

    nc = bacc.Bacc("TRN2", target_bir_lowering=False, debug=False, num_devices=N_CORES)
    xT = nc.dram_tensor("xT", [D_MODEL, C], f32r, kind="ExternalInput").ap()
    # w1 packed as [p, fb, k, fl]: w1p[p, fb, k, fl] = w1[k*128+p, fb*128+fl]
    w1p = nc.dram_tensor("w1p", [128, KF, KD, 128], f32r, kind="ExternalInput").ap()
    b1 = nc.dram_tensor("b1", [D_FF], f32, kind="ExternalInput").ap()
    # w2 packed as [p, db, kf, dl]: w2p[p, db, kf, dl] = w2[kf*128+p, db*128+dl]
    w2p = nc.dram_tensor("w2p", [128, KD, KF, 128], f32r, kind="ExternalInput").ap()
    b2 = nc.dram_tensor("b2", [D_MODEL], f32, kind="ExternalInput").ap()
    g = nc.dram_tensor("g", [C], f32, kind="ExternalInput").ap()
    yT = nc.dram_tensor("yT", [D_MODEL, C], f32, kind="ExternalOutput").ap()

    with tile.TileContext(nc) as tc, ExitStack() as ctx:
        const = ctx.enter_context(tc.tile_pool(name="const", bufs=1))
        xpool = ctx.enter_context(tc.tile_pool(name="x", bufs=KD + 1))
        hpool = ctx.enter_context(tc.tile_pool(name="h", bufs=KF))
        w1pool = ctx.enter_context(tc.tile_pool(name="w1pl", bufs=4))
        w2pool = ctx.enter_context(tc.tile_pool(name="w2pl", bufs=2))
        opool = ctx.enter_context(tc.tile_pool(name="o", bufs=4))
        ps1 = ctx.enter_context(tc.tile_pool(name="ps1", bufs=4, space="PSUM"))
        ps2 = ctx.enter_context(tc.tile_pool(name="ps2", bufs=4, space="PSUM"))

        # Constants: b1 as [128, 32] (col f = b1[f*128:(f+1)*128]), b2 as
        # [128, 8], g broadcast to all 128 partitions.
        b1_sb = const.tile([128, KF], f32)
        nc.sync.dma_start(b1_sb[:], b1.rearrange("(f p) -> p f", p=128))
        b2_sb = const.tile([128, KD], f32)
        nc.sync.dma_start(b2_sb[:], b2.rearrange("(d p) -> p d", p=128))
        g_sb = const.tile([128, C], f32)
        nc.sync.dma_start(g_sb[:], g.unsqueeze(0).partition_broadcast(128))

        for rep in range(reps):
          for S in range(n_super):
            tsl = slice(S * T_super, (S + 1) * T_super)
            xs = []
            for k in range(KD):
                xt = xpool.tile([128, T_super], f32r, tag="x", name=f"x_{rep}_{S}_{k}")
                nc.sync.dma_start(xt[:], xT[k * 128 : (k + 1) * 128, tsl])
                xs.append(xt)

            # ---- matmul 1: h[f, t] = relu(sum_k w1[k,f]^T x[k,t] + b1[f])
            hs = []
            for f in range(KF):
                w1c = w1pool.tile([128, KD, 128], f32r, tag="w1c", name=f"w1c_{rep}_{S}_{f}")
                nc.sync.dma_start(w1c[:], w1p[:, f])
                pst = [
                    ps1.tile([128, T_half], f32, tag="ps1", name=f"ps1_{rep}_{S}_{f}_{m}")
                    for m in range(2)
                ]
                for k in range(KD):
                    for m in range(2):
                        nc.tensor.matmul(
                            pst[m][:],
                            w1c[:, k, :],
                            xs[k][:, m * T_half : (m + 1) * T_half],
                            start=(k == 0),
                            stop=(k == KD - 1),
                        )
                ht = hpool.tile([128, T_super], f32r, tag="h", name=f"h_{rep}_{S}_{f}")
                for m in range(2):
                    nc.scalar.activation(
                        ht[:, m * T_half : (m + 1) * T_half],
                        pst[m][:],
                        Relu,
                        bias=b1_sb[:, f : f + 1],
                        scale=1.0,
                    )
                hs.append(ht)

            # ---- matmul 2: y[d, t] = (sum_f w2[f,d]^T h[f,t] + b2[d]) * g[t]
            for d in range(KD):
                w2c = w2pool.tile(
                    [128, KF, 128], f32r, tag="w2c", name=f"w2c_{rep}_{S}_{d}"
                )
                nc.scalar.dma_start(w2c[:], w2p[:, d])
                pst = [
                    ps2.tile([128, T_half], f32, tag="ps2", name=f"ps2_{rep}_{S}_{d}_{m}")
                    for m in range(2)
                ]
                for kf in range(KF):
                    for m in range(2):
                        nc.tensor.matmul(
                            pst[m][:],
                            w2c[:, kf, :],
                            hs[kf][:, m * T_half : (m + 1) * T_half],
                            start=(kf == 0),
                            stop=(kf == KF - 1),
                        )
                ot = opool.tile([128, T_super], f32, tag="o", name=f"o_{rep}_{S}_{d}")
                for m in range(2):
                    nc.vector.scalar_tensor_tensor(
                        ot[:, m * T_half : (m + 1) * T_half],
                        pst[m][:],
                        b2_sb[:, d : d + 1],
                        g_sb[:, S * T_super + m * T_half : S * T_super + (m + 1) * T_half],
                        op0=add,
                        op1=mult,
                    )
                nc.scalar.dma_start(yT[d * 128 : (d + 1) * 128, tsl], ot[:])

    nc.compile()
    return nc


def _get_program(C, T_super):
    key = (C, T_super)
    if key not in _PROGRAM_CACHE:
        _PROGRAM_CACHE[key] = _build_program(C, T_super)
    return _PROGRAM_CACHE[key]


def _route_host(x, gate_w, gate_b):
    """Router math mirroring the reference op-for-op (same jax eager ops in
    the same order -> matching top-k decisions)."""
    import jax
    import jax.numpy as jnp

    router_logits = jnp.einsum("bsd,de->bse", x, gate_w) + gate_b
    router_probs = jax.nn.softmax(router_logits, axis=-1)
    topk_probs, topk_idx = jax.lax.top_k(router_probs, TOP_K)
    topk_probs = topk_probs / jnp.sum(topk_probs, axis=-1, keepdims=True)

    one_hot = jax.nn.one_hot(topk_idx, NUM_EXPERTS, dtype=x.dtype)
    expert_usage = jnp.mean(one_hot, axis=(0, 1, 2))
    avg_probs = jnp.mean(router_probs, axis=(0, 1))
    load_balance_loss = jnp.var(avg_probs, ddof=1)
    return (
        np.asarray(topk_probs),
        np.asarray(topk_idx),
        np.asarray(expert_usage),
        np.asarray(load_balance_loss),
    )


def _pick_tiling(C_raw):
    T_super = 768
    n_super = max(1, -(-C_raw // T_super))
    return n_super * T_super, T_super


def _prepare_dispatch(x, topk_probs, topk_idx):
    """Per-expert gathered rows and gains."""
    x_flat = np.ascontiguousarray(np.asarray(x, dtype=np.float32).reshape(-1, D_MODEL))
    tk_idx = topk_idx.reshape(-1, TOP_K)
    tk_p = topk_probs.reshape(-1, TOP_K).astype(np.float32)

    rows_per_e = []
    gains_per_e = []
    for e in range(NUM_EXPERTS):
        sel = tk_idx == e
        rows = np.nonzero(sel.any(axis=1))[0]
        gains = (tk_p * sel)[rows].sum(axis=1, dtype=np.float32)
        rows_per_e.append(rows)
        gains_per_e.append(gains)
    return x_flat, rows_per_e, gains_per_e


def _build_in_maps(x_flat, rows_per_e, gains_per_e, w1, b1, w2, b2, C):
    w1_np = np.asarray(w1, dtype=np.float32)
    b1_np = np.asarray(b1, dtype=np.float32)
    w2_np = np.asarray(w2, dtype=np.float32)
    b2_np = np.asarray(b2, dtype=np.float32)

    in_maps = []
    for e in range(NUM_EXPERTS):
        rows = rows_per_e[e]
        xg = np.zeros((C, D_MODEL), dtype=np.float32)
        xg[: len(rows)] = x_flat[rows]
        gg = np.zeros((C,), dtype=np.float32)
        gg[: len(rows)] = gains_per_e[e]
        # w1p[p, fb, k, fl] = w1[k*128+p, fb*128+fl]
        w1p = np.ascontiguousarray(
            w1_np[e].reshape(KD, 128, KF, 128).transpose(1, 2, 0, 3)
        )
        # w2p[p, db, kf, dl] = w2[kf*128+p, db*128+dl]
        w2p = np.ascontiguousarray(
            w2_np[e].reshape(KF, 128, KD, 128).transpose(1, 2, 0, 3)
        )
        in_maps.append(
            {
                "xT": np.ascontiguousarray(xg.T),
                "w1p": w1p,
                "b1": b1_np[e],
                "w2p": w2p,
                "b2": b2_np[e],
                "g": gg,
            }
        )
    return in_maps


def kernel(x, gate_w, gate_b, w1, b1, w2, b2):
    from concourse.bass_utils import run_bass_kernel_spmd

    topk_probs, topk_idx, expert_usage, load_balance_loss = _route_host(
        x, gate_w, gate_b
    )
    x_flat, rows_per_e, gains_per_e = _prepare_dispatch(x, topk_probs, topk_idx)

    C_raw = max(len(r) for r in rows_per_e)
    C, T_super = _pick_tiling(C_raw)
    nc = _get_program(C, T_super)

    in_maps = _build_in_maps(x_flat, rows_per_e, gains_per_e, w1, b1, w2, b2, C)
    res = run_bass_kernel_spmd(nc, in_maps, list(range(N_CORES)))

    B, S, _ = np.asarray(x).shape
    out_flat = np.zeros((B * S, D_MODEL), dtype=np.float32)
    for e in range(NUM_EXPERTS):
        rows = rows_per_e[e]
        yT = res.results[e]["yT"]
        out_flat[rows] += yT[:, : len(rows)].T

    return (
        out_flat.reshape(B, S, D_MODEL),
        expert_usage.astype(np.float32),
        np.float32(load_balance_loss),
    )


# revision 10
# speedup vs baseline: 9.8496x; 1.0176x over previous
"""MoE (top-2 of 8 experts) Trainium2 kernel.

Strategy: expert-parallel across the 8 NeuronCores (1 expert per core).
The router (0.1% of the FLOPs) runs on the host with the exact op
sequence of the reference so top-k decisions match bitwise; the host
gathers each expert's tokens, each core runs that expert's FFN over its
tokens (feature-major layout), and the host scatter-adds the per-expert
outputs back into the full output.

Device kernel: h = relu(w1^T xT + b1); yT = (w2^T h + b2) * g, all
feature-major so both matmuls keep weights stationary and tokens
moving. Weights are host-repacked as [partition, block, k, in-block] so
every weight DMA is per-partition contiguous AND every stationary tile
is contiguous in SBUF.
"""

import numpy as np

D_MODEL = 1024
D_FF = 4096
NUM_EXPERTS = 8
TOP_K = 2
N_CORES = 8

KD = D_MODEL // 128  # 8 partition-tiles along d_model
KF = D_FF // 128  # 32 partition-tiles along d_ff

# dtype config: dtype for [x & w1 (matmul 1)] and [h & w2 (matmul 2)]
MM1_DT = "f32r"
MM2_DT = "f32r"
T_SUPER = 768
T_MOVE = 384

_PROGRAM_CACHE = {}


def _np_dt(name):
    if name == "bf16":
        import ml_dtypes

        return ml_dtypes.bfloat16
    return np.float32


def _build_program(C, T_super, reps=1, mm1_dt=None, mm2_dt=None):
    from contextlib import ExitStack

    import concourse.bacc as bacc
    import concourse.mybir as mybir
    import concourse.tile as tile

    mm1_dt = mm1_dt or MM1_DT
    mm2_dt = mm2_dt or MM2_DT
    f32 = mybir.dt.float32
    dt1 = {"f32r": mybir.dt.float32r, "bf16": mybir.dt.bfloat16}[mm1_dt]
    dt2 = {"f32r": mybir.dt.float32r, "bf16": mybir.dt.bfloat16}[mm2_dt]
    Relu = mybir.ActivationFunctionType.Relu
    add = mybir.AluOpType.add
    mult = mybir.AluOpType.mult

    assert C % T_super == 0 and T_super % T_MOVE == 0
    n_super = C // T_super
    n_m = T_super // T_MOVE
    TM = T_MOVE

    nc = bacc.Bacc("TRN2", target_bir_lowering=False, debug=False, num_devices=N_CORES)
    xT = nc.dram_tensor("xT", [D_MODEL, C], dt1, kind="ExternalInput").ap()
    # w1 packed as [p, fb, k, fl]: w1p[p, fb, k, fl] = w1[k*128+p, fb*128+fl]
    w1p = nc.dram_tensor("w1p", [128, KF, KD, 128], dt1, kind="ExternalInput").ap()
    b1 = nc.dram_tensor("b1", [128, KF], f32, kind="ExternalInput").ap()
    # w2 packed as [p, db, kf, dl]: w2p[p, db, kf, dl] = w2[kf*128+p, db*128+dl]
    w2p = nc.dram_tensor("w2p", [128, KD, KF, 128], dt2, kind="ExternalInput").ap()
    b2 = nc.dram_tensor("b2", [128, KD], f32, kind="ExternalInput").ap()
    g = nc.dram_tensor("g", [128, C], f32, kind="ExternalInput").ap()
    yT = nc.dram_tensor("yT", [D_MODEL, C], f32, kind="ExternalOutput").ap()

    with tile.TileContext(nc) as tc, ExitStack() as ctx:
        const = ctx.enter_context(tc.tile_pool(name="const", bufs=1))
        xpool = ctx.enter_context(tc.tile_pool(name="x", bufs=KD + 1))
        hpool = ctx.enter_context(tc.tile_pool(name="h", bufs=KF))
        w1pool = ctx.enter_context(tc.tile_pool(name="w1pl", bufs=4))
        w2pool = ctx.enter_context(tc.tile_pool(name="w2pl", bufs=2))
        opool = ctx.enter_context(tc.tile_pool(name="o", bufs=4))
        ps1 = ctx.enter_context(tc.tile_pool(name="ps1", bufs=min(2 * n_m, 4), space="PSUM"))
        ps2 = ctx.enter_context(tc.tile_pool(name="ps2", bufs=min(2 * n_m, 4), space="PSUM"))

        # Constants: b1 as [128, 32] (col f = b1[f*128:(f+1)*128]), b2 as
        # [128, 8], g broadcast to all 128 partitions.
        b1_sb = const.tile([128, KF], f32)
        nc.scalar.dma_start(b1_sb[:], b1[:])
        b2_sb = const.tile([128, KD], f32)
        nc.scalar.dma_start(b2_sb[:], b2[:])
        g_sb = const.tile([128, C], f32)
        nc.scalar.dma_start(g_sb[:], g[:])

        for rep in range(reps):
          for S in range(n_super):
            tsl = slice(S * T_super, (S + 1) * T_super)
            xs = []
            for k in range(KD):
                xt = xpool.tile([128, T_super], dt1, tag="x", name=f"x_{rep}_{S}_{k}")
                nc.sync.dma_start(xt[:], xT[k * 128 : (k + 1) * 128, tsl])
                xs.append(xt)

            # ---- matmul 1: h[f, t] = relu(sum_k w1[k,f]^T x[k,t] + b1[f])
            hs = []
            for f in range(KF):
                w1c = w1pool.tile([128, KD, 128], dt1, tag="w1c", name=f"w1c_{rep}_{S}_{f}")
                nc.sync.dma_start(w1c[:], w1p[:, f])
                pst = [
                    ps1.tile([128, TM], f32, tag="ps1", name=f"ps1_{rep}_{S}_{f}_{m}")
                    for m in range(n_m)
                ]
                for k in range(KD):
                    for m in range(n_m):
                        nc.tensor.matmul(
                            pst[m][:],
                            w1c[:, k, :],
                            xs[k][:, m * TM : (m + 1) * TM],
                            start=(k == 0),
                            stop=(k == KD - 1),
                        )
                ht = hpool.tile([128, T_super], dt2, tag="h", name=f"h_{rep}_{S}_{f}")
                for m in range(n_m):
                    nc.scalar.activation(
                        ht[:, m * TM : (m + 1) * TM],
                        pst[m][:],
                        Relu,
                        bias=b1_sb[:, f : f + 1],
                        scale=1.0,
                    )
                hs.append(ht)

            # ---- matmul 2: y[d, t] = (sum_f w2[f,d]^T h[f,t] + b2[d]) * g[t]
            for d in range(KD):
                w2c = w2pool.tile(
                    [128, KF, 128], dt2, tag="w2c", name=f"w2c_{rep}_{S}_{d}"
                )
                nc.scalar.dma_start(w2c[:], w2p[:, d])
                pst = [
                    ps2.tile([128, TM], f32, tag="ps2", name=f"ps2_{rep}_{S}_{d}_{m}")
                    for m in range(n_m)
                ]
                for kf in range(KF):
                    for m in range(n_m):
                        nc.tensor.matmul(
                            pst[m][:],
                            w2c[:, kf, :],
                            hs[kf][:, m * TM : (m + 1) * TM],
                            start=(kf == 0),
                            stop=(kf == KF - 1),
                        )
                ot = opool.tile([128, T_super], f32, tag="o", name=f"o_{rep}_{S}_{d}")
                for m in range(n_m):
                    nc.vector.scalar_tensor_tensor(
                        ot[:, m * TM : (m + 1) * TM],
                        pst[m][:],
                        b2_sb[:, d : d + 1],
                        g_sb[:, S * T_super + m * TM : S * T_super + (m + 1) * TM],
                        op0=add,
                        op1=mult,
                    )
                nc.scalar.dma_start(yT[d * 128 : (d + 1) * 128, tsl], ot[:])

    nc.compile()
    return nc


def _get_program(C, T_super):
    key = (C, T_super, MM1_DT, MM2_DT)
    if key not in _PROGRAM_CACHE:
        _PROGRAM_CACHE[key] = _build_program(C, T_super)
    return _PROGRAM_CACHE[key]


def _route_host(x, gate_w, gate_b):
    """Router math mirroring the reference op-for-op (same jax eager ops in
    the same order -> matching top-k decisions)."""
    import jax
    import jax.numpy as jnp

    router_logits = jnp.einsum("bsd,de->bse", x, gate_w) + gate_b
    router_probs = jax.nn.softmax(router_logits, axis=-1)
    topk_probs, topk_idx = jax.lax.top_k(router_probs, TOP_K)
    topk_probs = topk_probs / jnp.sum(topk_probs, axis=-1, keepdims=True)

    one_hot = jax.nn.one_hot(topk_idx, NUM_EXPERTS, dtype=x.dtype)
    expert_usage = jnp.mean(one_hot, axis=(0, 1, 2))
    avg_probs = jnp.mean(router_probs, axis=(0, 1))
    load_balance_loss = jnp.var(avg_probs, ddof=1)
    return (
        np.asarray(topk_probs),
        np.asarray(topk_idx),
        np.asarray(expert_usage),
        np.asarray(load_balance_loss),
    )


def _pick_tiling(C_raw):
    T_super = T_SUPER
    n_super = max(1, -(-C_raw // T_super))
    return n_super * T_super, T_super


def _prepare_dispatch(x, topk_probs, topk_idx):
    """Per-expert gathered rows and gains."""
    x_flat = np.ascontiguousarray(np.asarray(x, dtype=np.float32).reshape(-1, D_MODEL))
    tk_idx = topk_idx.reshape(-1, TOP_K)
    tk_p = topk_probs.reshape(-1, TOP_K).astype(np.float32)

    rows_per_e = []
    gains_per_e = []
    for e in range(NUM_EXPERTS):
        sel = tk_idx == e
        rows = np.nonzero(sel.any(axis=1))[0]
        gains = (tk_p * sel)[rows].sum(axis=1, dtype=np.float32)
        rows_per_e.append(rows)
        gains_per_e.append(gains)
    return x_flat, rows_per_e, gains_per_e


def _build_in_maps(x_flat, rows_per_e, gains_per_e, w1, b1, w2, b2, C):
    np1 = _np_dt(MM1_DT)
    np2 = _np_dt(MM2_DT)
    w1_np = np.asarray(w1, dtype=np.float32)
    b1_np = np.asarray(b1, dtype=np.float32)
    w2_np = np.asarray(w2, dtype=np.float32)
    b2_np = np.asarray(b2, dtype=np.float32)

    in_maps = []
    for e in range(NUM_EXPERTS):
        rows = rows_per_e[e]
        xg = np.zeros((C, D_MODEL), dtype=np.float32)
        xg[: len(rows)] = x_flat[rows]
        gg = np.zeros((C,), dtype=np.float32)
        gg[: len(rows)] = gains_per_e[e]
        # w1p[p, fb, k, fl] = w1[k*128+p, fb*128+fl]
        w1p = np.ascontiguousarray(
            w1_np[e].reshape(KD, 128, KF, 128).transpose(1, 2, 0, 3).astype(np1)
        )
        # w2p[p, db, kf, dl] = w2[kf*128+p, db*128+dl]
        w2p = np.ascontiguousarray(
            w2_np[e].reshape(KF, 128, KD, 128).transpose(1, 2, 0, 3).astype(np2)
        )
        in_maps.append(
            {
                "xT": np.ascontiguousarray(xg.T).astype(np1),
                "w1p": w1p,
                "b1": np.ascontiguousarray(b1_np[e].reshape(KF, 128).T),
                "w2p": w2p,
                "b2": np.ascontiguousarray(b2_np[e].reshape(KD, 128).T),
                "g": np.ascontiguousarray(np.broadcast_to(gg, (128, C))),
            }
        )
    return in_maps


def kernel(x, gate_w, gate_b, w1, b1, w2, b2):
    from concourse.bass_utils import run_bass_kernel_spmd

    topk_probs, topk_idx, expert_usage, load_balance_loss = _route_host(
        x, gate_w, gate_b
    )
    x_flat, rows_per_e, gains_per_e = _prepare_dispatch(x, topk_probs, topk_idx)

    C_raw = max(len(r) for r in rows_per_e)
    C, T_super = _pick_tiling(C_raw)
    nc = _get_program(C, T_super)

    in_maps = _build_in_maps(x_flat, rows_per_e, gains_per_e, w1, b1, w2, b2, C)
    res = run_bass_kernel_spmd(nc, in_maps, list(range(N_CORES)))

    B, S, _ = np.asarray(x).shape
    out_flat = np.zeros((B * S, D_MODEL), dtype=np.float32)
    for e in range(NUM_EXPERTS):
        rows = rows_per_e[e]
        yT = res.results[e]["yT"]
        out_flat[rows] += yT[:, : len(rows)].T

    return (
        out_flat.reshape(B, S, D_MODEL),
        expert_usage.astype(np.float32),
        np.float32(load_balance_loss),
    )


# revision 11
# speedup vs baseline: 9.9311x; 1.0083x over previous
"""MoE (top-2 of 8 experts) Trainium2 kernel.

Strategy: expert-parallel across the 8 NeuronCores (1 expert per core).
The router (0.1% of the FLOPs) runs on the host with the exact op
sequence of the reference so top-k decisions match bitwise; the host
gathers each expert's tokens, each core runs that expert's FFN over its
tokens (feature-major layout), and the host scatter-adds the per-expert
outputs back into the full output.

Device kernel: h = relu(w1^T xT + b1); yT = (w2^T h + b2) * g, all
feature-major so both matmuls keep weights stationary and tokens
moving. Weights are host-repacked as [partition, block, k, in-block] so
every weight DMA is per-partition contiguous AND every stationary tile
is contiguous in SBUF.
"""

import numpy as np

D_MODEL = 1024
D_FF = 4096
NUM_EXPERTS = 8
TOP_K = 2
N_CORES = 8

KD = D_MODEL // 128  # 8 partition-tiles along d_model
KF = D_FF // 128  # 32 partition-tiles along d_ff

# dtype config: dtype for [x & w1 (matmul 1)] and [h & w2 (matmul 2)]
MM1_DT = "f32r"
MM2_DT = "f32r"
T_SUPER = 768
T_MOVE = 384

_PROGRAM_CACHE = {}


def _np_dt(name):
    if name == "bf16":
        import ml_dtypes

        return ml_dtypes.bfloat16
    return np.float32


def _build_program(C, T_super, reps=1, mm1_dt=None, mm2_dt=None):
    from contextlib import ExitStack

    import concourse.bacc as bacc
    import concourse.mybir as mybir
    import concourse.tile as tile

    mm1_dt = mm1_dt or MM1_DT
    mm2_dt = mm2_dt or MM2_DT
    f32 = mybir.dt.float32
    dt1 = {"f32r": mybir.dt.float32r, "bf16": mybir.dt.bfloat16}[mm1_dt]
    dt2 = {"f32r": mybir.dt.float32r, "bf16": mybir.dt.bfloat16}[mm2_dt]
    Relu = mybir.ActivationFunctionType.Relu
    add = mybir.AluOpType.add
    mult = mybir.AluOpType.mult

    assert C % T_super == 0 and T_super % T_MOVE == 0
    n_super = C // T_super
    n_m = T_super // T_MOVE
    TM = T_MOVE

    nc = bacc.Bacc("TRN2", target_bir_lowering=False, debug=False, num_devices=N_CORES)
    xT = nc.dram_tensor("xT", [D_MODEL, C], dt1, kind="ExternalInput").ap()
    # w1 packed as [p, fb, k, fl]: w1p[p, fb, k, fl] = w1[k*128+p, fb*128+fl]
    w1p = nc.dram_tensor("w1p", [128, KF, KD, 128], dt1, kind="ExternalInput").ap()
    b1 = nc.dram_tensor("b1", [128, KF], f32, kind="ExternalInput").ap()
    # w2 packed as [p, db, kf, dl]: w2p[p, db, kf, dl] = w2[kf*128+p, db*128+dl]
    w2p = nc.dram_tensor("w2p", [128, KD, KF, 128], dt2, kind="ExternalInput").ap()
    b2 = nc.dram_tensor("b2", [128, KD], f32, kind="ExternalInput").ap()
    g = nc.dram_tensor("g", [128, C], f32, kind="ExternalInput").ap()
    yT = nc.dram_tensor("yT", [D_MODEL, C], f32, kind="ExternalOutput").ap()

    with tile.TileContext(nc) as tc, ExitStack() as ctx:
        const = ctx.enter_context(tc.tile_pool(name="const", bufs=1))
        xpool = ctx.enter_context(tc.tile_pool(name="x", bufs=KD + 1))
        hpool = ctx.enter_context(tc.tile_pool(name="h", bufs=KF))
        w1pool = ctx.enter_context(tc.tile_pool(name="w1pl", bufs=4))
        w2pool = ctx.enter_context(tc.tile_pool(name="w2pl", bufs=2))
        opool = ctx.enter_context(tc.tile_pool(name="o", bufs=4))
        ps1 = ctx.enter_context(tc.tile_pool(name="ps1", bufs=min(2 * n_m, 4), space="PSUM"))
        ps2 = ctx.enter_context(tc.tile_pool(name="ps2", bufs=min(2 * n_m, 4), space="PSUM"))

        # Constants: b1 as [128, 32] (col f = b1[f*128:(f+1)*128]), b2 as
        # [128, 8], g broadcast to all 128 partitions.
        b1_sb = const.tile([128, KF], f32)
        nc.scalar.dma_start(b1_sb[:], b1[:])
        b2_sb = const.tile([128, KD], f32)
        nc.scalar.dma_start(b2_sb[:], b2[:])
        g_sb = const.tile([128, C], f32)
        nc.scalar.dma_start(g_sb[:], g[:])

        for rep in range(reps):
          for S in range(n_super):
            tsl = slice(S * T_super, (S + 1) * T_super)
            first = rep == 0 and S == 0
            xs = []
            for k in range(KD):
                xt = xpool.tile([128, T_super], dt1, tag="x", name=f"x_{rep}_{S}_{k}")
                # On the very first super-tile, split x across both HWDGE
                # queues so the pipeline fills ~2x faster.
                eng = nc.scalar if (first and k % 2) else nc.sync
                eng.dma_start(xt[:], xT[k * 128 : (k + 1) * 128, tsl])
                xs.append(xt)

            # ---- matmul 1: h[f, t] = relu(sum_k w1[k,f]^T x[k,t] + b1[f])
            hs = []
            for f in range(KF):
                w1c = w1pool.tile([128, KD, 128], dt1, tag="w1c", name=f"w1c_{rep}_{S}_{f}")
                weng = nc.scalar if (first and f < 2) else nc.sync
                weng.dma_start(w1c[:], w1p[:, f])
                pst = [
                    ps1.tile([128, TM], f32, tag="ps1", name=f"ps1_{rep}_{S}_{f}_{m}")
                    for m in range(n_m)
                ]
                for k in range(KD):
                    for m in range(n_m):
                        nc.tensor.matmul(
                            pst[m][:],
                            w1c[:, k, :],
                            xs[k][:, m * TM : (m + 1) * TM],
                            start=(k == 0),
                            stop=(k == KD - 1),
                        )
                ht = hpool.tile([128, T_super], dt2, tag="h", name=f"h_{rep}_{S}_{f}")
                for m in range(n_m):
                    nc.scalar.activation(
                        ht[:, m * TM : (m + 1) * TM],
                        pst[m][:],
                        Relu,
                        bias=b1_sb[:, f : f + 1],
                        scale=1.0,
                    )
                hs.append(ht)

            # ---- matmul 2: y[d, t] = (sum_f w2[f,d]^T h[f,t] + b2[d]) * g[t]
            for d in range(KD):
                w2c = w2pool.tile(
                    [128, KF, 128], dt2, tag="w2c", name=f"w2c_{rep}_{S}_{d}"
                )
                nc.scalar.dma_start(w2c[:], w2p[:, d])
                pst = [
                    ps2.tile([128, TM], f32, tag="ps2", name=f"ps2_{rep}_{S}_{d}_{m}")
                    for m in range(n_m)
                ]
                for kf in range(KF):
                    for m in range(n_m):
                        nc.tensor.matmul(
                            pst[m][:],
                            w2c[:, kf, :],
                            hs[kf][:, m * TM : (m + 1) * TM],
                            start=(kf == 0),
                            stop=(kf == KF - 1),
                        )
                ot = opool.tile([128, T_super], f32, tag="o", name=f"o_{rep}_{S}_{d}")
                for m in range(n_m):
                    nc.vector.scalar_tensor_tensor(
                        ot[:, m * TM : (m + 1) * TM],
                        pst[m][:],
                        b2_sb[:, d : d + 1],
                        g_sb[:, S * T_super + m * TM : S * T_super + (m + 1) * TM],
                        op0=add,
                        op1=mult,
                    )
                nc.scalar.dma_start(yT[d * 128 : (d + 1) * 128, tsl], ot[:])

    nc.compile()
    return nc


def _get_program(C, T_super):
    key = (C, T_super, MM1_DT, MM2_DT)
    if key not in _PROGRAM_CACHE:
        _PROGRAM_CACHE[key] = _build_program(C, T_super)
    return _PROGRAM_CACHE[key]


def _route_host(x, gate_w, gate_b):
    """Router math mirroring the reference op-for-op (same jax eager ops in
    the same order -> matching top-k decisions)."""
    import jax
    import jax.numpy as jnp

    router_logits = jnp.einsum("bsd,de->bse", x, gate_w) + gate_b
    router_probs = jax.nn.softmax(router_logits, axis=-1)
    topk_probs, topk_idx = jax.lax.top_k(router_probs, TOP_K)
    topk_probs = topk_probs / jnp.sum(topk_probs, axis=-1, keepdims=True)

    one_hot = jax.nn.one_hot(topk_idx, NUM_EXPERTS, dtype=x.dtype)
    expert_usage = jnp.mean(one_hot, axis=(0, 1, 2))
    avg_probs = jnp.mean(router_probs, axis=(0, 1))
    load_balance_loss = jnp.var(avg_probs, ddof=1)
    return (
        np.asarray(topk_probs),
        np.asarray(topk_idx),
        np.asarray(expert_usage),
        np.asarray(load_balance_loss),
    )


def _pick_tiling(C_raw):
    T_super = T_SUPER
    n_super = max(1, -(-C_raw // T_super))
    return n_super * T_super, T_super


def _prepare_dispatch(x, topk_probs, topk_idx):
    """Per-expert gathered rows and gains."""
    x_flat = np.ascontiguousarray(np.asarray(x, dtype=np.float32).reshape(-1, D_MODEL))
    tk_idx = topk_idx.reshape(-1, TOP_K)
    tk_p = topk_probs.reshape(-1, TOP_K).astype(np.float32)

    rows_per_e = []
    gains_per_e = []
    for e in range(NUM_EXPERTS):
        sel = tk_idx == e
        rows = np.nonzero(sel.any(axis=1))[0]
        gains = (tk_p * sel)[rows].sum(axis=1, dtype=np.float32)
        rows_per_e.append(rows)
        gains_per_e.append(gains)
    return x_flat, rows_per_e, gains_per_e


def _build_in_maps(x_flat, rows_per_e, gains_per_e, w1, b1, w2, b2, C):
    np1 = _np_dt(MM1_DT)
    np2 = _np_dt(MM2_DT)
    w1_np = np.asarray(w1, dtype=np.float32)
    b1_np = np.asarray(b1, dtype=np.float32)
    w2_np = np.asarray(w2, dtype=np.float32)
    b2_np = np.asarray(b2, dtype=np.float32)

    in_maps = []
    for e in range(NUM_EXPERTS):
        rows = rows_per_e[e]
        xg = np.zeros((C, D_MODEL), dtype=np.float32)
        xg[: len(rows)] = x_flat[rows]
        gg = np.zeros((C,), dtype=np.float32)
        gg[: len(rows)] = gains_per_e[e]
        # w1p[p, fb, k, fl] = w1[k*128+p, fb*128+fl]
        w1p = np.ascontiguousarray(
            w1_np[e].reshape(KD, 128, KF, 128).transpose(1, 2, 0, 3).astype(np1)
        )
        # w2p[p, db, kf, dl] = w2[kf*128+p, db*128+dl]
        w2p = np.ascontiguousarray(
            w2_np[e].reshape(KF, 128, KD, 128).transpose(1, 2, 0, 3).astype(np2)
        )
        in_maps.append(
            {
                "xT": np.ascontiguousarray(xg.T).astype(np1),
                "w1p": w1p,
                "b1": np.ascontiguousarray(b1_np[e].reshape(KF, 128).T),
                "w2p": w2p,
                "b2": np.ascontiguousarray(b2_np[e].reshape(KD, 128).T),
                "g": np.ascontiguousarray(np.broadcast_to(gg, (128, C))),
            }
        )
    return in_maps


def kernel(x, gate_w, gate_b, w1, b1, w2, b2):
    from concourse.bass_utils import run_bass_kernel_spmd

    topk_probs, topk_idx, expert_usage, load_balance_loss = _route_host(
        x, gate_w, gate_b
    )
    x_flat, rows_per_e, gains_per_e = _prepare_dispatch(x, topk_probs, topk_idx)

    C_raw = max(len(r) for r in rows_per_e)
    C, T_super = _pick_tiling(C_raw)
    nc = _get_program(C, T_super)

    in_maps = _build_in_maps(x_flat, rows_per_e, gains_per_e, w1, b1, w2, b2, C)
    res = run_bass_kernel_spmd(nc, in_maps, list(range(N_CORES)))

    B, S, _ = np.asarray(x).shape
    out_flat = np.zeros((B * S, D_MODEL), dtype=np.float32)
    for e in range(NUM_EXPERTS):
        rows = rows_per_e[e]
        yT = res.results[e]["yT"]
        out_flat[rows] += yT[:, : len(rows)].T

    return (
        out_flat.reshape(B, S, D_MODEL),
        expert_usage.astype(np.float32),
        np.float32(load_balance_loss),
    )


# revision 12
# speedup vs baseline: 10.3746x; 1.0447x over previous
"""MoE (top-2 of 8 experts) Trainium2 kernel.

Strategy: expert-parallel across the 8 NeuronCores (1 expert per core).
The router (0.1% of the FLOPs) runs on the host with the exact op
sequence of the reference so top-k decisions match bitwise; the host
gathers each expert's tokens, each core runs that expert's FFN over its
tokens (feature-major layout), and the host scatter-adds the per-expert
outputs back into the full output.

Device kernel: h = relu(w1^T xT + b1); yT = (w2^T h + b2) * g, all
feature-major so both matmuls keep weights stationary and tokens
moving. Weights are host-repacked as [partition, block, k, in-block] so
every weight DMA is per-partition contiguous AND every stationary tile
is contiguous in SBUF.
"""

import numpy as np

D_MODEL = 1024
D_FF = 4096
NUM_EXPERTS = 8
TOP_K = 2
N_CORES = 8

KD = D_MODEL // 128  # 8 partition-tiles along d_model
KF = D_FF // 128  # 32 partition-tiles along d_ff

# dtype config: dtype for [x & w1 (matmul 1)] and [h & w2 (matmul 2)]
MM1_DT = "f32r"
MM2_DT = "f32r"
T_SUPER = 768
T_MOVE = 384

_PROGRAM_CACHE = {}


def _np_dt(name):
    if name == "bf16":
        import ml_dtypes

        return ml_dtypes.bfloat16
    return np.float32


def _build_program(C, supers, reps=1, mm1_dt=None, mm2_dt=None):
    from contextlib import ExitStack

    import concourse.bacc as bacc
    import concourse.mybir as mybir
    import concourse.tile as tile

    mm1_dt = mm1_dt or MM1_DT
    mm2_dt = mm2_dt or MM2_DT
    f32 = mybir.dt.float32
    dt1 = {"f32r": mybir.dt.float32r, "bf16": mybir.dt.bfloat16}[mm1_dt]
    dt2 = {"f32r": mybir.dt.float32r, "bf16": mybir.dt.bfloat16}[mm2_dt]
    Relu = mybir.ActivationFunctionType.Relu
    add = mybir.AluOpType.add
    mult = mybir.AluOpType.mult

    supers = tuple(supers)
    assert sum(supers) == C
    assert all(ts % 128 == 0 and ts >= 512 for ts in supers)
    n_super = len(supers)

    nc = bacc.Bacc("TRN2", target_bir_lowering=False, debug=False, num_devices=N_CORES)
    xT = nc.dram_tensor("xT", [D_MODEL, C], dt1, kind="ExternalInput").ap()
    # w1 packed as [p, fb, k, fl]: w1p[p, fb, k, fl] = w1[k*128+p, fb*128+fl]
    w1p = nc.dram_tensor("w1p", [128, KF, KD, 128], dt1, kind="ExternalInput").ap()
    b1 = nc.dram_tensor("b1", [128, KF], f32, kind="ExternalInput").ap()
    # w2 packed as [p, db, kf, dl]: w2p[p, db, kf, dl] = w2[kf*128+p, db*128+dl]
    w2p = nc.dram_tensor("w2p", [128, KD, KF, 128], dt2, kind="ExternalInput").ap()
    b2 = nc.dram_tensor("b2", [128, KD], f32, kind="ExternalInput").ap()
    g = nc.dram_tensor("g", [128, C], f32, kind="ExternalInput").ap()
    yT = nc.dram_tensor("yT", [D_MODEL, C], f32, kind="ExternalOutput").ap()

    with tile.TileContext(nc) as tc, ExitStack() as ctx:
        const = ctx.enter_context(tc.tile_pool(name="const", bufs=1))
        xpool = ctx.enter_context(tc.tile_pool(name="x", bufs=KD + 1))
        hpool = ctx.enter_context(tc.tile_pool(name="h", bufs=KF))
        w1pool = ctx.enter_context(tc.tile_pool(name="w1pl", bufs=4))
        w2pool = ctx.enter_context(tc.tile_pool(name="w2pl", bufs=2))
        opool = ctx.enter_context(tc.tile_pool(name="o", bufs=4))
        ps1 = ctx.enter_context(tc.tile_pool(name="ps1", bufs=4, space="PSUM"))
        ps2 = ctx.enter_context(tc.tile_pool(name="ps2", bufs=4, space="PSUM"))

        # Constants: b1 as [128, 32] (col f = b1[f*128:(f+1)*128]), b2 as
        # [128, 8], g broadcast to all 128 partitions.
        b1_sb = const.tile([128, KF], f32)
        nc.scalar.dma_start(b1_sb[:], b1[:])
        b2_sb = const.tile([128, KD], f32)
        nc.scalar.dma_start(b2_sb[:], b2[:])
        g_sb = const.tile([128, C], f32)
        nc.scalar.dma_start(g_sb[:], g[:])

        super_off = [sum(supers[:i]) for i in range(n_super)]
        for rep in range(reps):
          for S in range(n_super):
            T_super = supers[S]
            TM = T_super // 2
            n_m = 2
            tsl = slice(super_off[S], super_off[S] + T_super)
            first = rep == 0 and S == 0
            xs = []
            for k in range(KD):
                xt = xpool.tile([128, T_super], dt1, tag="x", name=f"x_{rep}_{S}_{k}")
                # On the very first super-tile, split x across both HWDGE
                # queues so the pipeline fills ~2x faster.
                eng = nc.scalar if (first and k % 2) else nc.sync
                eng.dma_start(xt[:], xT[k * 128 : (k + 1) * 128, tsl])
                xs.append(xt)

            # ---- matmul 1: h[f, t] = relu(sum_k w1[k,f]^T x[k,t] + b1[f])
            hs = []
            for f in range(KF):
                w1c = w1pool.tile([128, KD, 128], dt1, tag="w1c", name=f"w1c_{rep}_{S}_{f}")
                weng = nc.scalar if (first and f < 2) else nc.sync
                weng.dma_start(w1c[:], w1p[:, f])
                pst = [
                    ps1.tile([128, TM], f32, tag="ps1", name=f"ps1_{rep}_{S}_{f}_{m}")
                    for m in range(n_m)
                ]
                for k in range(KD):
                    for m in range(n_m):
                        nc.tensor.matmul(
                            pst[m][:],
                            w1c[:, k, :],
                            xs[k][:, m * TM : (m + 1) * TM],
                            start=(k == 0),
                            stop=(k == KD - 1),
                        )
                ht = hpool.tile([128, T_super], dt2, tag="h", name=f"h_{rep}_{S}_{f}")
                for m in range(n_m):
                    nc.scalar.activation(
                        ht[:, m * TM : (m + 1) * TM],
                        pst[m][:],
                        Relu,
                        bias=b1_sb[:, f : f + 1],
                        scale=1.0,
                    )
                hs.append(ht)

            # ---- matmul 2: y[d, t] = (sum_f w2[f,d]^T h[f,t] + b2[d]) * g[t]
            for d in range(KD):
                w2c = w2pool.tile(
                    [128, KF, 128], dt2, tag="w2c", name=f"w2c_{rep}_{S}_{d}"
                )
                nc.scalar.dma_start(w2c[:], w2p[:, d])
                pst = [
                    ps2.tile([128, TM], f32, tag="ps2", name=f"ps2_{rep}_{S}_{d}_{m}")
                    for m in range(n_m)
                ]
                for kf in range(KF):
                    for m in range(n_m):
                        nc.tensor.matmul(
                            pst[m][:],
                            w2c[:, kf, :],
                            hs[kf][:, m * TM : (m + 1) * TM],
                            start=(kf == 0),
                            stop=(kf == KF - 1),
                        )
                ot = opool.tile([128, T_super], f32, tag="o", name=f"o_{rep}_{S}_{d}")
                for m in range(n_m):
                    nc.vector.scalar_tensor_tensor(
                        ot[:, m * TM : (m + 1) * TM],
                        pst[m][:],
                        b2_sb[:, d : d + 1],
                        g_sb[:, super_off[S] + m * TM : super_off[S] + (m + 1) * TM],
                        op0=add,
                        op1=mult,
                    )
                nc.scalar.dma_start(yT[d * 128 : (d + 1) * 128, tsl], ot[:])

    nc.compile()
    return nc


def _get_program(C, supers):
    key = (C, tuple(supers), MM1_DT, MM2_DT)
    if key not in _PROGRAM_CACHE:
        _PROGRAM_CACHE[key] = _build_program(C, supers)
    return _PROGRAM_CACHE[key]


def _route_host(x, gate_w, gate_b):
    """Router math mirroring the reference op-for-op (same jax eager ops in
    the same order -> matching top-k decisions)."""
    import jax
    import jax.numpy as jnp

    router_logits = jnp.einsum("bsd,de->bse", x, gate_w) + gate_b
    router_probs = jax.nn.softmax(router_logits, axis=-1)
    topk_probs, topk_idx = jax.lax.top_k(router_probs, TOP_K)
    topk_probs = topk_probs / jnp.sum(topk_probs, axis=-1, keepdims=True)

    one_hot = jax.nn.one_hot(topk_idx, NUM_EXPERTS, dtype=x.dtype)
    expert_usage = jnp.mean(one_hot, axis=(0, 1, 2))
    avg_probs = jnp.mean(router_probs, axis=(0, 1))
    load_balance_loss = jnp.var(avg_probs, ddof=1)
    return (
        np.asarray(topk_probs),
        np.asarray(topk_idx),
        np.asarray(expert_usage),
        np.asarray(load_balance_loss),
    )


def _pick_tiling(C_raw):
    """Split C (C_raw rounded up to 64) into supertiles of 512..832 tokens,
    each a multiple of 128 (so the two moving halves are multiples of 64 and
    >= 256 for full-rate float32r)."""
    C = max(512, -(-C_raw // 64) * 64)
    n = max(1, -(-C // 832))
    # make C divisible into n parts of multiples of 128 (m-halves mult of 64)
    C = max(C, 512 * n)
    C = -(-C // 128) * 128
    base = C // n // 128 * 128
    rem = (C - base * n) // 128
    supers = [base + 128 * (1 if i < rem else 0) for i in range(n)]
    assert sum(supers) == C and all(512 <= ts <= 960 for ts in supers), supers
    return C, supers


def _prepare_dispatch(x, topk_probs, topk_idx):
    """Per-expert gathered rows and gains."""
    x_flat = np.ascontiguousarray(np.asarray(x, dtype=np.float32).reshape(-1, D_MODEL))
    tk_idx = topk_idx.reshape(-1, TOP_K)
    tk_p = topk_probs.reshape(-1, TOP_K).astype(np.float32)

    rows_per_e = []
    gains_per_e = []
    for e in range(NUM_EXPERTS):
        sel = tk_idx == e
        rows = np.nonzero(sel.any(axis=1))[0]
        gains = (tk_p * sel)[rows].sum(axis=1, dtype=np.float32)
        rows_per_e.append(rows)
        gains_per_e.append(gains)
    return x_flat, rows_per_e, gains_per_e


def _build_in_maps(x_flat, rows_per_e, gains_per_e, w1, b1, w2, b2, C):
    np1 = _np_dt(MM1_DT)
    np2 = _np_dt(MM2_DT)
    w1_np = np.asarray(w1, dtype=np.float32)
    b1_np = np.asarray(b1, dtype=np.float32)
    w2_np = np.asarray(w2, dtype=np.float32)
    b2_np = np.asarray(b2, dtype=np.float32)

    in_maps = []
    for e in range(NUM_EXPERTS):
        rows = rows_per_e[e]
        xg = np.zeros((C, D_MODEL), dtype=np.float32)
        xg[: len(rows)] = x_flat[rows]
        gg = np.zeros((C,), dtype=np.float32)
        gg[: len(rows)] = gains_per_e[e]
        # w1p[p, fb, k, fl] = w1[k*128+p, fb*128+fl]
        w1p = np.ascontiguousarray(
            w1_np[e].reshape(KD, 128, KF, 128).transpose(1, 2, 0, 3).astype(np1)
        )
        # w2p[p, db, kf, dl] = w2[kf*128+p, db*128+dl]
        w2p = np.ascontiguousarray(
            w2_np[e].reshape(KF, 128, KD, 128).transpose(1, 2, 0, 3).astype(np2)
        )
        in_maps.append(
            {
                "xT": np.ascontiguousarray(xg.T).astype(np1),
                "w1p": w1p,
                "b1": np.ascontiguousarray(b1_np[e].reshape(KF, 128).T),
                "w2p": w2p,
                "b2": np.ascontiguousarray(b2_np[e].reshape(KD, 128).T),
                "g": np.ascontiguousarray(np.broadcast_to(gg, (128, C))),
            }
        )
    return in_maps


def kernel(x, gate_w, gate_b, w1, b1, w2, b2):
    from concourse.bass_utils import run_bass_kernel_spmd

    topk_probs, topk_idx, expert_usage, load_balance_loss = _route_host(
        x, gate_w, gate_b
    )
    x_flat, rows_per_e, gains_per_e = _prepare_dispatch(x, topk_probs, topk_idx)

    C_raw = max(len(r) for r in rows_per_e)
    C, supers = _pick_tiling(C_raw)
    nc = _get_program(C, supers)

    in_maps = _build_in_maps(x_flat, rows_per_e, gains_per_e, w1, b1, w2, b2, C)
    res = run_bass_kernel_spmd(nc, in_maps, list(range(N_CORES)))

    B, S, _ = np.asarray(x).shape
    out_flat = np.zeros((B * S, D_MODEL), dtype=np.float32)
    for e in range(NUM_EXPERTS):
        rows = rows_per_e[e]
        yT = res.results[e]["yT"]
        out_flat[rows] += yT[:, : len(rows)].T

    return (
        out_flat.reshape(B, S, D_MODEL),
        expert_usage.astype(np.float32),
        np.float32(load_balance_loss),
    )
